# revision 2
# baseline (speedup 1.0000x reference)
import numpy as np
import ml_dtypes

import concourse.bacc as bacc
import concourse.bass as bass
import concourse.mybir as mybir
import concourse.tile as tile
from concourse import bass_utils

bf16 = ml_dtypes.bfloat16
f8 = ml_dtypes.float8_e4m3

B, N, D = 4, 2048, 1024
NQ, NK = 1024, 2048
FP32 = mybir.dt.float32
BF16 = mybir.dt.bfloat16
FP8 = mybir.dt.float8e4
EXP = mybir.ActivationFunctionType.Exp
SQRT = mybir.ActivationFunctionType.Sqrt
DR = mybir.MatmulPerfMode.DoubleRow

WS = 64.0          # weight scale for fp8
RS = WS * WS       # residual prescale = 4096
LN_EPS = 1e-5 * RS * RS

LAST_EXEC_NS = None
_NC = None


def _broadcast_ap(dram_ap, parts):
    return bass.AP(
        tensor=dram_ap.tensor,
        offset=dram_ap.offset,
        ap=[[0, parts], dram_ap.ap[-1]],
    )


def _build():
    nc = bacc.Bacc(None, target_bir_lowering=False)
    qT = nc.dram_tensor("qT", [D, NQ], FP8, kind="ExternalInput")
    kT = nc.dram_tensor("kT", [D, NK], FP8, kind="ExternalInput")
    vT = nc.dram_tensor("vT", [D, NK], FP8, kind="ExternalInput")
    wq = nc.dram_tensor("wq", [D, D], FP8, kind="ExternalInput")
    wk = nc.dram_tensor("wk", [D, D], FP8, kind="ExternalInput")
    wv = nc.dram_tensor("wv", [D, D], FP8, kind="ExternalInput")
    wo = nc.dram_tensor("wo", [D, D], FP8, kind="ExternalInput")
    qn = nc.dram_tensor("qn", [NQ, D], FP32, kind="ExternalInput")
    gamma = nc.dram_tensor("gamma", [1, D], FP32, kind="ExternalInput")
    beta = nc.dram_tensor("beta", [1, D], FP32, kind="ExternalInput")
    out = nc.dram_tensor("out", [NQ, D], FP32, kind="ExternalOutput")

    stage_re = "(d2 j p) n -> p d2 j n"

    with tile.TileContext(nc) as tc:
        with (
            tc.tile_pool(name="perm", bufs=1) as perm,
            tc.tile_pool(name="ps", bufs=1, space="PSUM") as ps,
        ):
            gamma_t = perm.tile([128, D], BF16)
            beta_t = perm.tile([128, D], BF16)
            nc.gpsimd.dma_start(out=gamma_t, in_=_broadcast_ap(gamma[0:1, :], 128))
            nc.gpsimd.dma_start(out=beta_t, in_=_broadcast_ap(beta[0:1, :], 128))
            eps_t = perm.tile([128, 1], FP32)
            nc.vector.memset(eps_t, LN_EPS)
            ones_f = perm.tile([128, 64], FP32)
            nc.vector.memset(ones_f, 1.0)
            ones_b = perm.tile([128, 1], BF16)
            nc.vector.memset(ones_b, 1.0)

            wq_t = perm.tile([128, 4, 2, D], FP8)
            wk_t = perm.tile([128, 4, 2, D], FP8)
            wv_t = perm.tile([128, 4, 2, D], FP8)
            wo_t = perm.tile([128, 4, 2, D], FP8)
            qstage = perm.tile([128, 4, 2, NQ], FP8)
            kstages = [perm.tile([128, 4, 2, 512], FP8, name=f"kst{i}") for i in range(4)]
            vstages = [perm.tile([128, 4, 2, 256], FP8, tag="vst", bufs=2, name=f"vst{i}") for i in range(8)]

            nc.sync.dma_start(wq_t, wq[:, :].rearrange(stage_re, p=128, j=2))
            nc.sync.dma_start(qstage, qT[:, :].rearrange(stage_re, p=128, j=2))
            nc.sync.dma_start(wk_t, wk[:, :].rearrange(stage_re, p=128, j=2))
            nc.sync.dma_start(
                kstages[0], kT[:, 0:512].rearrange(stage_re, p=128, j=2))
            nc.sync.dma_start(wv_t, wv[:, :].rearrange(stage_re, p=128, j=2))
            nc.sync.dma_start(
                vstages[0], vT[:, 0:256].rearrange(stage_re, p=128, j=2))
            nc.sync.dma_start(
                vstages[1], vT[:, 256:512].rearrange(stage_re, p=128, j=2))
            for i in range(1, 4):
                nc.sync.dma_start(
                    kstages[i], kT[:, i * 512:(i + 1) * 512].rearrange(stage_re, p=128, j=2)
                )
            for i in range(2, 8):
                nc.sync.dma_start(
                    vstages[i], vT[:, i * 256:(i + 1) * 256].rearrange(stage_re, p=128, j=2)
                )
            nc.sync.dma_start(wo_t, wo[:, :].rearrange(stage_re, p=128, j=2))

            qs_t = [perm.tile([128, NQ], BF16, name=f"qs{j}") for j in range(8)]
            ksT = [perm.tile([128, NK], BF16, name=f"ks{j}") for j in range(8)]
            vsp = [perm.tile([128, 1024], BF16, name=f"vsp{t}") for t in range(16)]
            at_q = [perm.tile([128, 8, 512], FP8, name=f"atq{i}") for i in range(2)]

            with tc.tile_pool(name="work", bufs=1) as wk_pool:

                def qproj_group(jt, qh):
                    pp = ps.tile([128, 512], FP32, tag="pp", bufs=2, name="pp_q")
                    for d2 in range(4):
                        nc.tensor.matmul(
                            pp,
                            wq_t[:, d2, :, jt * 128:(jt + 1) * 128],
                            qstage[:, d2, :, qh * 512:(qh + 1) * 512],
                            start=(d2 == 0), stop=(d2 == 3),
                            perf_mode=DR,
                        )
                    nc.vector.tensor_copy(qs_t[jt][:, qh * 512:(qh + 1) * 512], pp)

                def kproj_group(jt, kc):
                    pp = ps.tile([128, 512], FP32, tag="pp", bufs=2, name="pp_k")
                    for d2 in range(4):
                        nc.tensor.matmul(
                            pp,
                            wk_t[:, d2, :, jt * 128:(jt + 1) * 128],
                            kstages[kc][:, d2, :, :],
                            start=(d2 == 0), stop=(d2 == 3),
                            perf_mode=DR,
                        )
                    nc.vector.tensor_copy(ksT[jt][:, kc * 512:(kc + 1) * 512], pp)

                def vproj_group(kt_i, jc):
                    rc, rt = divmod(kt_i, 2)
                    pp = ps.tile([128, 512], FP32, tag="pp", bufs=2, name="pp_v")
                    for d2 in range(4):
                        nc.tensor.matmul(
                            pp,
                            vstages[rc][:, d2, :, rt * 128:(rt + 1) * 128],
                            wv_t[:, d2, :, jc * 512:(jc + 1) * 512],
                            start=(d2 == 0), stop=(d2 == 3),
                            perf_mode=DR,
                        )
                    nc.vector.tensor_copy(vsp[kt_i][:, jc * 512:(jc + 1) * 512], pp)

                oproj_state = {}

                def oproj_open(qc):
                    outf = [wk_pool.tile([128, D], FP32, tag=f"outf{qc}_{rt}", bufs=1,
                                         name=f"outf{qc}_{rt}") for rt in range(4)]
                    mvall = wk_pool.tile([128, 4, 2], FP32, tag="mvall", bufs=2)
                    oproj_state[qc] = (outf, mvall)

                def oproj_rt(qc, rt):
                    outf, mvall = oproj_state[qc]
                    row0 = qc * 512 + rt * 128
                    of = outf[rt]
                    nc.sync.dma_start(of, qn[row0:row0 + 128, :])
                    for oc in range(2):
                        po = ps.tile([128, 512], FP32, tag="pp", bufs=2, name="pp_o")
                        for h2 in (3, 2, 1, 0):
                            nc.tensor.matmul(
                                po,
                                at_q[qc][:, h2 * 2:h2 * 2 + 2, rt * 128:(rt + 1) * 128],
                                wo_t[:, h2, :, oc * 512:(oc + 1) * 512],
                                start=(h2 == 3), stop=(h2 == 0),
                                perf_mode=DR,
                            )
                        nc.vector.tensor_add(
                            out=of[:, oc * 512:(oc + 1) * 512],
                            in0=of[:, oc * 512:(oc + 1) * 512], in1=po,
                        )
                    bst = wk_pool.tile([128, 2, 6], FP32, tag="bst", bufs=1)
                    for sg in range(2):
                        nc.vector.bn_stats(out=bst[:, sg, :], in_=of[:, sg * 512:(sg + 1) * 512])
                    nc.vector.bn_aggr(out=mvall[:, rt, :], in_=bst)

                def ln_finish(qc):
                    outf, mvall = oproj_state.pop(qc)
                    nc.scalar.activation(
                        out=mvall[:, :, 1:2], in_=mvall[:, :, 1:2], func=SQRT,
                        bias=eps_t[:, :], scale=1.0,
                    )
                    nc.vector.reciprocal(mvall[:, :, 1:2], mvall[:, :, 1:2])
                    for rt in range(4):
                        row0 = qc * 512 + rt * 128
                        y = wk_pool.tile([128, D], FP32, tag="y", bufs=2)
                        nc.vector.tensor_scalar(
                            out=y, in0=outf[rt],
                            scalar1=mvall[:, rt, 0:1], scalar2=mvall[:, rt, 1:2],
                            op0=mybir.AluOpType.subtract, op1=mybir.AluOpType.mult,
                        )
                        nc.vector.tensor_mul(y, y, gamma_t)
                        nc.vector.tensor_add(out=y, in0=y, in1=beta_t)
                        nc.sync.dma_start(out[row0:row0 + 128, :], y)

                # ---- filler schedules: (slot, thunk) per (qc, hp) ----
                fillers = {(qc, hp): [] for qc in range(2) for hp in range(8)}
                # hp0/qc0: V projection kt 2..15 emitted two tiles ahead,
                # plus remaining K-proj blocks of head-pair 0
                for kt_i in range(2, 16):
                    for jc in range(2):
                        fillers[(0, 0)].append(
                            (kt_i - 2, lambda kt_i=kt_i, jc=jc: vproj_group(kt_i, jc)))
                for kc in range(1, 4):
                    fillers[(0, 0)].append(
                        (4 * kc - 2, lambda kc=kc: kproj_group(0, kc)))
                # head-pair hp (qc0): kc0+qproj during previous hp, kc1..3 in own loop
                for hp in range(1, 8):
                    fillers[(0, hp - 1)].append(
                        (5, lambda hp=hp: kproj_group(hp, 0)))
                    fillers[(0, hp - 1)].append(
                        (9, lambda hp=hp: qproj_group(hp, 0)))
                    for kc in range(1, 4):
                        fillers[(0, hp)].append(
                            (4 * kc - 3, lambda hp=hp, kc=kc: kproj_group(hp, kc)))
                # qc1 q-projections (qh=1): first during last qc0 loop, rest rolling
                fillers[(0, 7)].append((8, lambda: qproj_group(7, 1)))
                fillers[(0, 7)].append((12, lambda: qproj_group(6, 1)))
                for i in range(2, 8):
                    fillers[(1, i - 2)].append(
                        (7, lambda i=i: qproj_group(7 - i, 1)))
                # out-projection of qc0 spread over qc1
                fillers[(1, 0)].append((10, lambda: oproj_open(0)))
                for rt in range(4):
                    fillers[(1, rt + 1)].append(
                        (2, lambda rt=rt: oproj_rt(0, rt)))
                fillers[(1, 5)].append((2, lambda: ln_finish(0)))

                # ---- prelude: minimum needed for (qc0, hp0, kt0..1) ----
                qproj_group(0, 0)
                kproj_group(0, 0)
                for kt_i in range(2):
                    for jc in range(2):
                        vproj_group(kt_i, jc)

                # ---- attention ----
                for qc in range(2):
                    qsl = slice(qc * 512, (qc + 1) * 512)
                    pending = [None]

                    def emit_norm(p, qc=qc):
                        pv_sb, dn_sb, hp = p
                        rcp = wk_pool.tile([128, 512], FP32, tag="rcp", bufs=1)
                        nc.vector.reciprocal_approx_fast(rcp[0:33, :], dn_sb[0:33, :])
                        brec = ps.tile([128, 512], FP32, tag="pp", bufs=2, name="brec")
                        nc.tensor.matmul(
                            brec[0:64, :], ones_f[0:1, 0:64], rcp[0:1, :],
                            start=True, stop=True, tile_position=(0, 0),
                            skip_group_check=True,
                        )
                        nc.tensor.matmul(
                            brec[64:128, :], ones_f[32:33, 0:64], rcp[32:33, :],
                            start=True, stop=True, tile_position=(32, 64),
                            skip_group_check=True,
                        )
                        nc.vector.tensor_mul(at_q[qc][:, hp, :], pv_sb, brec)

                    hp_order = range(8) if qc == 0 else range(7, -1, -1)
                    for pos, hp in enumerate(hp_order):
                        filler = sorted(fillers[(qc, pos)], key=lambda x: x[0])
                        fi = 0
                        pv = ps.tile([128, 512], FP32, tag="pv", bufs=1)
                        dn = ps.tile([128, 512], FP32, tag="dn", bufs=1)
                        nc.vector.memset(dn, 1.0)
                        for kt in range(16):
                            sc = ps.tile([128, 1024], FP32, tag="sc", bufs=2)
                            ktb = slice(kt * 128, (kt + 1) * 128)
                            nc.tensor.matmul(
                                sc[:, 0:512], ksT[hp][0:64, ktb], qs_t[hp][0:64, qsl],
                                start=True, stop=True, skip_group_check=True,
                            )
                            nc.tensor.matmul(
                                sc[:, 512:1024], ksT[hp][64:128, ktb], qs_t[hp][64:128, qsl],
                                start=True, stop=True,
                                tile_position=(64, 0), skip_group_check=True,
                            )
                            E = wk_pool.tile([128, 1024], BF16, tag="E", bufs=3)
                            nc.scalar.activation(E, sc, func=EXP, bias=0.0, scale=0.125 / RS)
                            nc.tensor.matmul(
                                pv[0:64, :], vsp[kt][:, hp * 128:hp * 128 + 64],
                                E[:, 0:512],
                                start=(kt == 0), stop=(kt == 15),
                                tile_position=(0, 0), skip_group_check=True,
                            )
                            nc.tensor.matmul(
                                pv[64:128, :], vsp[kt][:, hp * 128 + 64:hp * 128 + 128],
                                E[:, 512:1024],
                                start=(kt == 0), stop=(kt == 15),
                                tile_position=(0, 64), skip_group_check=True,
                            )
                            for j, st in enumerate([E[:, 0:512], E[:, 512:1024]]):
                                nc.tensor.matmul(
                                    dn[32 * j:32 * j + 1, :], ones_b[:, 0:1], st,
                                    start=False, stop=(kt == 15),
                                    tile_position=(0, 32 * j),
                                    skip_group_check=True,
                                )
                            while fi < len(filler) and filler[fi][0] <= kt:
                                filler[fi][1]()
                                fi += 1
                        while fi < len(filler):
                            filler[fi][1]()
                            fi += 1
                        pv_sb = wk_pool.tile([128, 512], BF16, tag="pvsb", bufs=2)
                        nc.vector.tensor_copy(pv_sb, pv)
                        dn_sb = wk_pool.tile([128, 512], FP32, tag="dnsb", bufs=2)
                        nc.vector.tensor_copy(dn_sb[0:33, :], dn[0:33, :])
                        if pending[0] is not None:
                            emit_norm(pending[0])
                        pending[0] = (pv_sb, dn_sb, hp)
                    emit_norm(pending[0])

                # ---- tail: out-projection + LN for qc1 ----
                oproj_open(1)
                for rt in range(4):
                    oproj_rt(1, rt)
                ln_finish(1)
    nc.finalize()
    return nc


def kernel(q, k, v, Wq, Wk, Wv, Wo, gamma, beta, _trace=False):
    global _NC, LAST_EXEC_NS
    if _NC is None:
        _NC = _build()
    wqh = np.ascontiguousarray(Wq.T * WS).astype(f8)
    wkh = np.ascontiguousarray(Wk.T * WS).astype(f8)
    wvh = np.ascontiguousarray(Wv.T * WS).astype(f8)
    woh = np.ascontiguousarray(Wo.T * WS).astype(f8)
    g = np.ascontiguousarray(np.asarray(gamma, dtype=np.float32).reshape(1, D))
    bt = np.ascontiguousarray(np.asarray(beta, dtype=np.float32).reshape(1, D))
    in_maps = []
    for c in range(8):
        b, hh = divmod(c, 2)
        qb = q[b, hh * NQ:(hh + 1) * NQ, :]
        in_maps.append({
            "qT": np.ascontiguousarray(qb.T).astype(f8),
            "qn": np.ascontiguousarray(qb, dtype=np.float32) * RS,
            "kT": np.ascontiguousarray(k[b].T).astype(f8),
            "vT": np.ascontiguousarray(v[b].T).astype(f8),
            "wq": wqh, "wk": wkh, "wv": wvh, "wo": woh,
            "gamma": g, "beta": bt,
        })
    res = bass_utils.run_bass_kernel_spmd(_NC, in_maps, list(range(8)), trace=_trace)
    LAST_EXEC_NS = getattr(res, "exec_time_ns", None)
    outp = np.empty((B, N, D), np.float32)
    for c in range(8):
        b, hh = divmod(c, 2)
        outp[b, hh * NQ:(hh + 1) * NQ, :] = res.results[c]["out"]
    return outp
